# revision 1
# baseline (speedup 1.0000x reference)
"""Trainium2 Bass kernel for an additive-coupling (NICE-style) flow layer.

Math (mask_config=0, forward, reverse=False):
    perm = perm_bank[arm_index]
    x1 = x[:, perm[0::2]]; x2 = x[:, perm[1::2]]
    m  = MLP(x1)  (Linear-ReLU-Linear-ReLU-Linear, D/2 -> 128 -> 128 -> D/2)
    y[:, j] = x[:, j]                 where inv_perm[j] even
    y[:, perm[2i+1]] = x[:, perm[2i+1]] + m[:, i]

The permutation gather/scatter is folded into zero-padded weight matrices on
the host, so the device kernel is a pure dense MLP over x plus an elementwise
add:  y = x + relu(relu(x @ W0p.T + b0) @ W1.T + b1) @ W2s.T + b2s
with W0p [128, D] (zero cols at odd-perm positions) and W2s.T [128, D] (zero
cols at even-perm positions).  Sharding: data parallel over batch, 8 cores.
"""

import numpy as np

B = 32768
D = 2048
MID = 128
N_CORES = 8
BS = B // N_CORES        # 4096 rows per core
P = 128                  # partitions
KC = D // P              # 16 contraction chunks for layer 1
NT = BS // P             # 32 row tiles per core

_CACHE: dict = {}


def _build_nc(use_b2: bool):
    import concourse.mybir as mybir
    import concourse.tile as tile
    from concourse import bacc

    dt = mybir.dt.float32
    Relu = mybir.ActivationFunctionType.Relu

    nc = bacc.Bacc("TRN2", target_bir_lowering=False, debug=False,
                   num_devices=N_CORES)

    x_d = nc.dram_tensor("x", (BS, D), dt, kind="ExternalInput").ap()
    w0_d = nc.dram_tensor("w0", (P, D), dt, kind="ExternalInput").ap()
    w1_d = nc.dram_tensor("w1t", (P, P), dt, kind="ExternalInput").ap()
    w2_d = nc.dram_tensor("w2st", (P, D), dt, kind="ExternalInput").ap()
    b0_d = nc.dram_tensor("b0", (P, 1), dt, kind="ExternalInput").ap()
    b1_d = nc.dram_tensor("b1", (P, 1), dt, kind="ExternalInput").ap()
    id_d = nc.dram_tensor("ident", (P, P), dt, kind="ExternalInput").ap()
    if use_b2:
        b2_d = nc.dram_tensor("b2s", (1, D), dt, kind="ExternalInput").ap()
        on_d = nc.dram_tensor("ones", (1, P), dt, kind="ExternalInput").ap()
    y_d = nc.dram_tensor("y", (BS, D), dt, kind="ExternalOutput").ap()

    with tile.TileContext(nc) as tc:
        with (
            tc.tile_pool(name="consts", bufs=1) as cp,
            tc.tile_pool(name="xp", bufs=3) as xp,
            tc.tile_pool(name="xtp", bufs=2) as xtp,
            tc.tile_pool(name="hp", bufs=2) as hp,
            tc.tile_pool(name="pxt", bufs=3, space="PSUM") as pxt,
            tc.tile_pool(name="ph0", bufs=2, space="PSUM") as ph0,
            tc.tile_pool(name="ph1", bufs=1, space="PSUM") as ph1,
            tc.tile_pool(name="pm", bufs=2, space="PSUM") as pm,
        ):
            w0_sb = cp.tile([P, D], dt)
            nc.sync.dma_start(out=w0_sb, in_=w0_d)
            w1_sb = cp.tile([P, P], dt)
            nc.sync.dma_start(out=w1_sb, in_=w1_d)
            w2_sb = cp.tile([P, D], dt)
            nc.sync.dma_start(out=w2_sb, in_=w2_d)
            b0_sb = cp.tile([P, 1], dt)
            nc.sync.dma_start(out=b0_sb, in_=b0_d)
            b1_sb = cp.tile([P, 1], dt)
            nc.sync.dma_start(out=b1_sb, in_=b1_d)
            id_sb = cp.tile([P, P], dt)
            nc.sync.dma_start(out=id_sb, in_=id_d)
            if use_b2:
                b2_sb = cp.tile([1, D], dt)
                nc.sync.dma_start(out=b2_sb, in_=b2_d)
                on_sb = cp.tile([1, P], dt)
                nc.sync.dma_start(out=on_sb, in_=on_d)

            for t in range(NT):
                rows = slice(t * P, (t + 1) * P)
                x_sb = xp.tile([P, D], dt)
                nc.sync.dma_start(out=x_sb, in_=x_d[rows, :])

                # Transpose x tile chunk-by-chunk via PE; evict PSUM->SBUF in
                # [P, 512] groups, alternating DVE/ACT.
                xt_sb = xtp.tile([P, D], dt)
                for j in range(KC // 4):
                    pxt_t = pxt.tile([P, 512], dt)
                    for u in range(4):
                        k = 4 * j + u
                        nc.tensor.transpose(
                            pxt_t[:, u * P:(u + 1) * P],
                            x_sb[:, k * P:(k + 1) * P],
                            id_sb,
                        )
                    dst = xt_sb[:, j * 512:(j + 1) * 512]
                    if j % 2 == 0:
                        nc.vector.tensor_copy(out=dst, in_=pxt_t)
                    else:
                        nc.scalar.copy(dst, pxt_t)

                # Layer 1: h0T[mid, b] = sum_k W0pT_k.T @ xT_k  (K=D chunks)
                ph0_t = ph0.tile([P, P], dt)
                for k in range(KC):
                    nc.tensor.matmul(
                        ph0_t,
                        w0_sb[:, k * P:(k + 1) * P],
                        xt_sb[:, k * P:(k + 1) * P],
                        start=(k == 0),
                        stop=(k == KC - 1),
                    )
                h0_sb = hp.tile([P, P], dt, tag="h0")
                nc.scalar.activation(h0_sb, ph0_t, Relu, bias=b0_sb[:, :])

                # Layer 2
                ph1_t = ph1.tile([P, P], dt)
                nc.tensor.matmul(ph1_t, w1_sb, h0_sb)
                h1_sb = hp.tile([P, P], dt, tag="h1")
                nc.scalar.activation(h1_sb, ph1_t, Relu, bias=b1_sb[:, :])

                # Layer 3 (+ residual add into x tile, in place), then store.
                for n in range(D // 512):
                    cols = slice(n * 512, (n + 1) * 512)
                    pm_t = pm.tile([P, 512], dt)
                    if use_b2:
                        nc.tensor.matmul(pm_t, on_sb, b2_sb[:, cols],
                                         start=True, stop=False)
                        nc.tensor.matmul(pm_t, h1_sb, w2_sb[:, cols],
                                         start=False, stop=True)
                    else:
                        nc.tensor.matmul(pm_t, h1_sb, w2_sb[:, cols])
                    nc.vector.tensor_add(out=x_sb[:, cols], in0=x_sb[:, cols],
                                         in1=pm_t)
                nc.sync.dma_start(out=y_d[rows, :], in_=x_sb)

    nc.compile()
    return nc


def _prep_host(W0, b0, W1, b1, W2, b2, perm):
    """Fold the permutation gather/scatter into padded weight matrices."""
    f32 = np.float32
    E = perm[0::2]
    O = perm[1::2]
    half = D // 2
    W0 = np.asarray(W0, f32)      # [MID, half]
    W1 = np.asarray(W1, f32)      # [MID, MID]
    W2 = np.asarray(W2, f32)      # [half, MID]

    W0pT = np.zeros((D, MID), f32)
    W0pT[E, :] = W0.T             # [D, MID]; row d used iff d in even-perm set
    # SBUF layout: w0[p, k*128+m] = W0pT[k*128+p, m]
    w0_host = np.ascontiguousarray(
        W0pT.reshape(KC, P, MID).transpose(1, 0, 2).reshape(P, KC * MID))

    w1t_host = np.ascontiguousarray(W1.T)

    w2st_host = np.zeros((MID, D), f32)
    w2st_host[:, O] = W2.T        # columns at odd-perm positions carry W2

    b2s_host = np.zeros((1, D), f32)
    b2s_host[0, O] = np.asarray(b2, f32)

    return dict(
        w0=w0_host,
        w1t=w1t_host,
        w2st=w2st_host,
        b0=np.ascontiguousarray(np.asarray(b0, f32).reshape(P, 1)),
        b1=np.ascontiguousarray(np.asarray(b1, f32).reshape(P, 1)),
        ident=np.eye(P, dtype=f32),
        b2s=b2s_host,
        ones=np.ones((1, P), f32),
    )


def _get_nc(use_b2: bool):
    key = ("nc", use_b2)
    if key not in _CACHE:
        _CACHE[key] = _build_nc(use_b2)
    return _CACHE[key]


def kernel(x, log_det_J, W0, b0, W1, b1, W2, b2, perm_bank, inv_perm_bank,
           arm_index):
    from concourse.bass_utils import run_bass_kernel_spmd

    x = np.ascontiguousarray(np.asarray(x, np.float32))
    perm = np.asarray(perm_bank)[int(arm_index)].astype(np.int64)

    consts = _prep_host(W0, b0, W1, b1, W2, b2, perm)
    use_b2 = bool(np.any(consts["b2s"]))
    if not use_b2:
        consts.pop("b2s")
        consts.pop("ones")

    nc = _get_nc(use_b2)

    in_maps = []
    for c in range(N_CORES):
        m = {"x": np.ascontiguousarray(x[c * BS:(c + 1) * BS])}
        m.update(consts)
        in_maps.append(m)

    res = run_bass_kernel_spmd(nc, in_maps, core_ids=list(range(N_CORES)))
    y = np.concatenate([res.results[c]["y"] for c in range(N_CORES)], axis=0)
    return y, np.asarray(log_det_J, np.float32)


# revision 4
# speedup vs baseline: 12.8479x; 12.8479x over previous
"""Trainium2 Bass kernel for an additive-coupling (NICE-style) flow layer.

Math (mask_config=0, forward, reverse=False):
    perm = perm_bank[arm_index]
    x1 = x[:, perm[0::2]]; x2 = x[:, perm[1::2]]
    m  = MLP(x1)  (Linear-ReLU-Linear-ReLU-Linear, D/2 -> 128 -> 128 -> D/2)
    y[:, j] = x[:, j]                 where inv_perm[j] even
    y[:, perm[2i+1]] = x[:, perm[2i+1]] + m[:, i]

The permutation gather/scatter is folded into zero-padded weight matrices on
the host, so the device kernel is a pure dense MLP over x plus an elementwise
add:  y = x + relu(relu(x @ W0p.T + b0) @ W1.T + b1) @ W2s.T + b2s
with W0p [128, D] (zero cols at odd-perm positions) and W2s.T [128, D] (zero
cols at even-perm positions).  Sharding: data parallel over batch, 8 cores.
"""

import numpy as np

B = 32768
D = 2048
MID = 128
N_CORES = 8
BS = B // N_CORES        # 4096 rows per core
P = 128                  # partitions
KC = D // P              # 16 contraction chunks for layer 1
NT = BS // P             # 32 row tiles per core

_CACHE: dict = {}


def _build_nc(use_b2: bool, repeats: int = 1):
    import concourse.mybir as mybir
    import concourse.tile as tile
    from concourse import bacc

    dt = mybir.dt.float32
    Relu = mybir.ActivationFunctionType.Relu

    nc = bacc.Bacc("TRN2", target_bir_lowering=False, debug=False,
                   num_devices=N_CORES)

    x_d = nc.dram_tensor("x", (BS, D), dt, kind="ExternalInput").ap()
    w0_d = nc.dram_tensor("w0", (P, D), dt, kind="ExternalInput").ap()
    w1_d = nc.dram_tensor("w1t", (P, P), dt, kind="ExternalInput").ap()
    w2_d = nc.dram_tensor("w2st", (P, D), dt, kind="ExternalInput").ap()
    b0_d = nc.dram_tensor("b0", (P, 1), dt, kind="ExternalInput").ap()
    b1_d = nc.dram_tensor("b1", (P, 1), dt, kind="ExternalInput").ap()
    id_d = nc.dram_tensor("ident", (P, P), dt, kind="ExternalInput").ap()
    if use_b2:
        b2_d = nc.dram_tensor("b2s", (1, D), dt, kind="ExternalInput").ap()
        on_d = nc.dram_tensor("ones", (1, P), dt, kind="ExternalInput").ap()
    y_d = nc.dram_tensor("y", (BS, D), dt, kind="ExternalOutput").ap()

    with tile.TileContext(nc) as tc:
        with (
            tc.tile_pool(name="consts", bufs=1) as cp,
            tc.tile_pool(name="xp", bufs=3) as xp,
            tc.tile_pool(name="xtp", bufs=2) as xtp,
            tc.tile_pool(name="hp", bufs=2) as hp,
            tc.tile_pool(name="pxt", bufs=3, space="PSUM") as pxt,
            tc.tile_pool(name="ph0", bufs=2, space="PSUM") as ph0,
            tc.tile_pool(name="ph1", bufs=1, space="PSUM") as ph1,
            tc.tile_pool(name="pm", bufs=2, space="PSUM") as pm,
        ):
            w0_sb = cp.tile([P, D], dt)
            nc.sync.dma_start(out=w0_sb, in_=w0_d)
            w1_sb = cp.tile([P, P], dt)
            nc.sync.dma_start(out=w1_sb, in_=w1_d)
            w2_sb = cp.tile([P, D], dt)
            nc.sync.dma_start(out=w2_sb, in_=w2_d)
            b0_sb = cp.tile([P, 1], dt)
            nc.sync.dma_start(out=b0_sb, in_=b0_d)
            b1_sb = cp.tile([P, 1], dt)
            nc.sync.dma_start(out=b1_sb, in_=b1_d)
            id_sb = cp.tile([P, P], dt)
            nc.sync.dma_start(out=id_sb, in_=id_d)
            if use_b2:
                b2_sb = cp.tile([1, D], dt)
                nc.sync.dma_start(out=b2_sb, in_=b2_d)
                on_sb = cp.tile([1, P], dt)
                nc.sync.dma_start(out=on_sb, in_=on_d)

            for t in range(NT * repeats):
                t = t % NT
                rows = slice(t * P, (t + 1) * P)
                x_sb = xp.tile([P, D], dt)
                nc.sync.dma_start(out=x_sb, in_=x_d[rows, :])

                # Transpose x tile chunk-by-chunk via PE; evict PSUM->SBUF in
                # [P, 512] groups, alternating DVE/ACT.
                xt_sb = xtp.tile([P, D], dt)
                for j in range(KC // 4):
                    pxt_t = pxt.tile([P, 512], dt)
                    for u in range(4):
                        k = 4 * j + u
                        nc.tensor.transpose(
                            pxt_t[:, u * P:(u + 1) * P],
                            x_sb[:, k * P:(k + 1) * P],
                            id_sb,
                        )
                    dst = xt_sb[:, j * 512:(j + 1) * 512]
                    if j % 2 == 0:
                        nc.vector.tensor_copy(out=dst, in_=pxt_t)
                    else:
                        nc.scalar.copy(dst, pxt_t)

                # Layer 1: h0T[mid, b] = sum_k W0pT_k.T @ xT_k  (K=D chunks)
                ph0_t = ph0.tile([P, P], dt)
                for k in range(KC):
                    nc.tensor.matmul(
                        ph0_t,
                        w0_sb[:, k * P:(k + 1) * P],
                        xt_sb[:, k * P:(k + 1) * P],
                        start=(k == 0),
                        stop=(k == KC - 1),
                    )
                h0_sb = hp.tile([P, P], dt, tag="h0")
                nc.scalar.activation(h0_sb, ph0_t, Relu, bias=b0_sb[:, :])

                # Layer 2
                ph1_t = ph1.tile([P, P], dt)
                nc.tensor.matmul(ph1_t, w1_sb, h0_sb)
                h1_sb = hp.tile([P, P], dt, tag="h1")
                nc.scalar.activation(h1_sb, ph1_t, Relu, bias=b1_sb[:, :])

                # Layer 3 (+ residual add into x tile, in place), then store.
                for n in range(D // 512):
                    cols = slice(n * 512, (n + 1) * 512)
                    pm_t = pm.tile([P, 512], dt)
                    if use_b2:
                        nc.tensor.matmul(pm_t, on_sb, b2_sb[:, cols],
                                         start=True, stop=False)
                        nc.tensor.matmul(pm_t, h1_sb, w2_sb[:, cols],
                                         start=False, stop=True)
                    else:
                        nc.tensor.matmul(pm_t, h1_sb, w2_sb[:, cols])
                    nc.vector.tensor_add(out=x_sb[:, cols], in0=x_sb[:, cols],
                                         in1=pm_t)
                nc.sync.dma_start(out=y_d[rows, :], in_=x_sb)

    nc.compile()
    return nc


def _prep_host(W0, b0, W1, b1, W2, b2, perm):
    """Fold the permutation gather/scatter into padded weight matrices."""
    f32 = np.float32
    E = perm[0::2]
    O = perm[1::2]
    half = D // 2
    W0 = np.asarray(W0, f32)      # [MID, half]
    W1 = np.asarray(W1, f32)      # [MID, MID]
    W2 = np.asarray(W2, f32)      # [half, MID]

    W0pT = np.zeros((D, MID), f32)
    W0pT[E, :] = W0.T             # [D, MID]; row d used iff d in even-perm set
    # SBUF layout: w0[p, k*128+m] = W0pT[k*128+p, m]
    w0_host = np.ascontiguousarray(
        W0pT.reshape(KC, P, MID).transpose(1, 0, 2).reshape(P, KC * MID))

    w1t_host = np.ascontiguousarray(W1.T)

    w2st_host = np.zeros((MID, D), f32)
    w2st_host[:, O] = W2.T        # columns at odd-perm positions carry W2

    b2s_host = np.zeros((1, D), f32)
    b2s_host[0, O] = np.asarray(b2, f32)

    return dict(
        w0=w0_host,
        w1t=w1t_host,
        w2st=w2st_host,
        b0=np.ascontiguousarray(np.asarray(b0, f32).reshape(P, 1)),
        b1=np.ascontiguousarray(np.asarray(b1, f32).reshape(P, 1)),
        ident=np.eye(P, dtype=f32),
        b2s=b2s_host,
        ones=np.ones((1, P), f32),
    )


def _get_nc(use_b2: bool, repeats: int = 1):
    key = ("nc", use_b2, repeats)
    if key not in _CACHE:
        _CACHE[key] = _build_nc(use_b2, repeats)
    return _CACHE[key]


def kernel(x, log_det_J, W0, b0, W1, b1, W2, b2, perm_bank, inv_perm_bank,
           arm_index):
    from concourse.bass_utils import run_bass_kernel_spmd

    x = np.ascontiguousarray(np.asarray(x, np.float32))
    perm = np.asarray(perm_bank)[int(arm_index)].astype(np.int64)

    consts = _prep_host(W0, b0, W1, b1, W2, b2, perm)
    use_b2 = bool(np.any(consts["b2s"]))
    if not use_b2:
        consts.pop("b2s")
        consts.pop("ones")

    nc = _get_nc(use_b2)

    in_maps = []
    for c in range(N_CORES):
        m = {"x": np.ascontiguousarray(x[c * BS:(c + 1) * BS])}
        m.update(consts)
        in_maps.append(m)

    res = run_bass_kernel_spmd(nc, in_maps, core_ids=list(range(N_CORES)))
    y = np.concatenate([res.results[c]["y"] for c in range(N_CORES)], axis=0)
    return y, np.asarray(log_det_J, np.float32)


# revision 13
# speedup vs baseline: 14.2449x; 1.1087x over previous
"""Trainium2 Bass kernel for an additive-coupling (NICE-style) flow layer.

Math (mask_config=0, forward, reverse=False):
    perm = perm_bank[arm_index]
    x1 = x[:, perm[0::2]]; x2 = x[:, perm[1::2]]
    m  = MLP(x1)  (Linear-ReLU-Linear-ReLU-Linear, D/2 -> 128 -> 128 -> D/2)
    y[:, j] = x[:, j]                 where inv_perm[j] even
    y[:, perm[2i+1]] = x[:, perm[2i+1]] + m[:, i]

The permutation gather/scatter is folded into zero-padded weight matrices on
the host, so the device kernel is a pure dense MLP over x plus an elementwise
add:  y = x + relu(relu(x @ W0p.T + b0) @ W1.T + b1) @ W2s.T + b2s
with W0p [128, D] (zero cols at odd-perm positions) and W2s.T [128, D] (zero
cols at even-perm positions).  Sharding: data parallel over batch, 8 cores.
"""

import numpy as np

B = 32768
D = 2048
MID = 128
N_CORES = 8
BS = B // N_CORES        # 4096 rows per core
P = 128                  # partitions
KC = D // P              # 16 contraction chunks for layer 1
NT = BS // P             # 32 row tiles per core

_CACHE: dict = {}


def _build_nc(use_b2: bool, repeats: int = 1):
    import concourse.mybir as mybir
    import concourse.tile as tile
    from concourse import bacc

    dt = mybir.dt.float32
    Relu = mybir.ActivationFunctionType.Relu

    nc = bacc.Bacc("TRN2", target_bir_lowering=False, debug=False,
                   num_devices=N_CORES)

    dtr = mybir.dt.float32r
    x_d = nc.dram_tensor("x", (BS, D), dt, kind="ExternalInput").ap()
    w0_d = nc.dram_tensor("w0", (P, D), dtr, kind="ExternalInput").ap()
    w1_d = nc.dram_tensor("w1t", (P, P), dtr, kind="ExternalInput").ap()
    w2_d = nc.dram_tensor("w2st", (P, D), dtr, kind="ExternalInput").ap()
    b0_d = nc.dram_tensor("b0", (P, 1), dt, kind="ExternalInput").ap()
    b1_d = nc.dram_tensor("b1", (P, 1), dt, kind="ExternalInput").ap()
    id_d = nc.dram_tensor("ident", (P, P), dt, kind="ExternalInput").ap()
    if use_b2:
        b2_d = nc.dram_tensor("b2s", (1, D), dtr, kind="ExternalInput").ap()
        on_d = nc.dram_tensor("ones", (1, P), dtr, kind="ExternalInput").ap()
    y_d = nc.dram_tensor("y", (BS, D), dt, kind="ExternalOutput").ap()
    BB = 512                 # batch rows per block (4 row tiles)
    RT = BB // P             # row tiles per block
    NB = BS // BB            # blocks per core

    with tile.TileContext(nc) as tc:
        with (
            tc.tile_pool(name="consts", bufs=1) as cp,
            tc.tile_pool(name="xp", bufs=2) as xp,
            tc.tile_pool(name="xtp", bufs=2) as xtp,
            tc.tile_pool(name="hp", bufs=2) as hp,
            tc.tile_pool(name="pxt", bufs=3, space="PSUM") as pxt,
            tc.tile_pool(name="ph0", bufs=1, space="PSUM") as ph0,
            tc.tile_pool(name="ph1", bufs=1, space="PSUM") as ph1,
            tc.tile_pool(name="pm", bufs=3, space="PSUM") as pm,
        ):
            w0_sb = cp.tile([P, D], dtr)
            nc.sync.dma_start(out=w0_sb, in_=w0_d)
            w1_sb = cp.tile([P, P], dtr)
            nc.sync.dma_start(out=w1_sb, in_=w1_d)
            w2_sb = cp.tile([P, D], dtr)
            nc.sync.dma_start(out=w2_sb, in_=w2_d)
            b0_sb = cp.tile([P, 1], dt)
            nc.sync.dma_start(out=b0_sb, in_=b0_d)
            b1_sb = cp.tile([P, 1], dt)
            nc.sync.dma_start(out=b1_sb, in_=b1_d)
            id_sb = cp.tile([P, P], dt)
            nc.sync.dma_start(out=id_sb, in_=id_d)
            if use_b2:
                b2_sb = cp.tile([1, D], dtr)
                nc.sync.dma_start(out=b2_sb, in_=b2_d)
                on_sb = cp.tile([1, P], dtr)
                nc.sync.dma_start(out=on_sb, in_=on_d)

            for blk in range(NB * repeats):
                blk = blk % NB
                x_sb = xp.tile([P, RT * D], dt)
                for rt in range(RT):
                    rows = slice(blk * BB + rt * P, blk * BB + (rt + 1) * P)
                    nc.sync.dma_start(out=x_sb[:, rt * D:(rt + 1) * D],
                                      in_=x_d[rows, :])

                # Transpose the block via PE. Chunk k of xt is
                # [128 feats, 512 batch rows]; evict PSUM->SBUF (rounding to
                # f32r for the matmuls) in [P, 512] groups, alternating
                # DVE/ACT.
                xt_sb = xtp.tile([P, KC * BB], dtr)
                for k in range(KC):
                    pxt_t = pxt.tile([P, BB], dt)
                    for rt in range(RT):
                        nc.tensor.transpose(
                            pxt_t[:, rt * P:(rt + 1) * P],
                            x_sb[:, rt * D + k * P: rt * D + (k + 1) * P],
                            id_sb,
                        )
                    dst = xt_sb[:, k * BB:(k + 1) * BB]
                    if k % 2 == 0:
                        nc.vector.tensor_copy(out=dst, in_=pxt_t)
                    else:
                        nc.scalar.copy(dst, pxt_t)

                # Layer 1: h0T[mid, b] = sum_k W0pT_k.T @ xT_k  (K=D chunks)
                ph0_t = ph0.tile([P, BB], dt)
                for k in range(KC):
                    nc.tensor.matmul(
                        ph0_t,
                        w0_sb[:, k * P:(k + 1) * P],
                        xt_sb[:, k * BB:(k + 1) * BB],
                        start=(k == 0),
                        stop=(k == KC - 1),
                    )
                h0_sb = hp.tile([P, BB], dtr, tag="h0")
                nc.scalar.activation(h0_sb, ph0_t, Relu, bias=b0_sb[:, :])

                # Layer 2
                ph1_t = ph1.tile([P, BB], dt)
                nc.tensor.matmul(ph1_t, w1_sb, h0_sb)
                h1_sb = hp.tile([P, BB], dtr, tag="h1")
                nc.scalar.activation(h1_sb, ph1_t, Relu, bias=b1_sb[:, :])

                # Layer 3 (+ residual add into x tile, in place), then store.
                for rt in range(RT):
                    for n in range(D // 512):
                        cols = slice(rt * D + n * 512, rt * D + (n + 1) * 512)
                        pm_t = pm.tile([P, 512], dt)
                        lhs = h1_sb[:, rt * P:(rt + 1) * P]
                        rhs = w2_sb[:, n * 512:(n + 1) * 512]
                        if use_b2:
                            nc.tensor.matmul(pm_t, on_sb,
                                             b2_sb[:, n * 512:(n + 1) * 512],
                                             start=True, stop=False)
                            nc.tensor.matmul(pm_t, lhs, rhs,
                                             start=False, stop=True)
                        else:
                            nc.tensor.matmul(pm_t, lhs, rhs)
                        nc.vector.tensor_add(out=x_sb[:, cols],
                                             in0=x_sb[:, cols], in1=pm_t)
                    rows = slice(blk * BB + rt * P, blk * BB + (rt + 1) * P)
                    nc.sync.dma_start(out=y_d[rows, :],
                                      in_=x_sb[:, rt * D:(rt + 1) * D])

    nc.compile()
    return nc


def _prep_host(W0, b0, W1, b1, W2, b2, perm):
    """Fold the permutation gather/scatter into padded weight matrices."""
    f32 = np.float32
    E = perm[0::2]
    O = perm[1::2]
    half = D // 2
    W0 = np.asarray(W0, f32)      # [MID, half]
    W1 = np.asarray(W1, f32)      # [MID, MID]
    W2 = np.asarray(W2, f32)      # [half, MID]

    W0pT = np.zeros((D, MID), f32)
    W0pT[E, :] = W0.T             # [D, MID]; row d used iff d in even-perm set
    # SBUF layout: w0[p, k*128+m] = W0pT[k*128+p, m]
    w0_host = np.ascontiguousarray(
        W0pT.reshape(KC, P, MID).transpose(1, 0, 2).reshape(P, KC * MID))

    w1t_host = np.ascontiguousarray(W1.T)

    w2st_host = np.zeros((MID, D), f32)
    w2st_host[:, O] = W2.T        # columns at odd-perm positions carry W2

    b2s_host = np.zeros((1, D), f32)
    b2s_host[0, O] = np.asarray(b2, f32)

    return dict(
        w0=w0_host,
        w1t=w1t_host,
        w2st=w2st_host,
        b0=np.ascontiguousarray(np.asarray(b0, f32).reshape(P, 1)),
        b1=np.ascontiguousarray(np.asarray(b1, f32).reshape(P, 1)),
        ident=np.eye(P, dtype=f32),
        b2s=b2s_host,
        ones=np.ones((1, P), f32),
    )


def _get_nc(use_b2: bool, repeats: int = 1):
    key = ("nc", use_b2, repeats)
    if key not in _CACHE:
        _CACHE[key] = _build_nc(use_b2, repeats)
    return _CACHE[key]


def kernel(x, log_det_J, W0, b0, W1, b1, W2, b2, perm_bank, inv_perm_bank,
           arm_index):
    from concourse.bass_utils import run_bass_kernel_spmd

    x = np.ascontiguousarray(np.asarray(x, np.float32))
    perm = np.asarray(perm_bank)[int(arm_index)].astype(np.int64)

    consts = _prep_host(W0, b0, W1, b1, W2, b2, perm)
    use_b2 = bool(np.any(consts["b2s"]))
    if not use_b2:
        consts.pop("b2s")
        consts.pop("ones")

    nc = _get_nc(use_b2)

    in_maps = []
    for c in range(N_CORES):
        m = {"x": np.ascontiguousarray(x[c * BS:(c + 1) * BS])}
        m.update(consts)
        in_maps.append(m)

    res = run_bass_kernel_spmd(nc, in_maps, core_ids=list(range(N_CORES)))
    y = np.concatenate([res.results[c]["y"] for c in range(N_CORES)], axis=0)
    return y, np.asarray(log_det_J, np.float32)
